# revision 42
# baseline (speedup 1.0000x reference)
"""Self-contained Trainium2 Bass kernel for nn_Attention_41472204210330.

Multi-head attention (B=2, T=2048, HIDDEN=1024, 16 heads, head_dim=64, fp32)
with RoPE, sharded over 8 NeuronCores: data-parallel over the batch (2) x
tensor-parallel over heads (4 groups of 4 heads).  Each core computes its
batch's q/k/v projections for its 4 heads, RoPE, attention, and a partial
output projection (its heads' slice of wo); the host sums the 4 partials per
batch element.

Performance design (v2): the PE p-state/HAM clock only stays at full speed
(2.4 GHz) while the PE instruction stream is gap-free, so the whole kernel is
emitted as ONE dense PE schedule:
  - xT is DMA'd in 16 [128,1024] pieces in priority order so the first
    projection matmul issues ~1.6us in; everything else preloads behind it.
  - RoPE: the head-dim axis is interleaved host-side ([d0,d32,d1,d33,...]) so
    rotate-half becomes an intra-32 partition pair swap == one DVE
    stream_shuffle (no SBUF-SBUF DMA bounce).  cos/sin tables are bf16 with
    the sign pre-baked.
  - attends are ACT(exp)-bound by ~25%; a master filler sequence (v tiles,
    m=1 q/k projections, and the output projection split into per-head-pair
    accumulation passes A/B) drips PE work into every attend window.
  - softmax denominator via a ones-column in V; normalization is
    reciprocal([1,T] direct from PSUM) + gpsimd partition_broadcast (no DRAM
    bounce).
  - yT partials are stored in bf16 (host sums in fp32).
"""

import sys

if "/opt/trn_rl_repo" not in sys.path:
    sys.path.insert(0, "/opt/trn_rl_repo")

import numpy as np

import bass_rust
import concourse.bass as bass
import concourse.mybir as mybir
import concourse.tile as tile

HIDDEN = 1024
NUM_HEADS = 16
D = 64  # head dim
B = 2
T = 2048
N_CORES = 8
HPC = NUM_HEADS // (N_CORES // B)  # heads per core = 4
HD = HPC * D  # per-core head dims = 256
P = 128
F32 = mybir.dt.float32
BF16 = mybir.dt.bfloat16
NQ = 1024  # q-half width
IC_CH = HIDDEN // P  # 8 input-channel chunks
NKT = T // P  # 16 k tiles
VW = D + 1  # v columns per head incl. ones column


def _split_waits(nc):
    """The in-container walrus caps semaphore waits per instruction lower
    than bass_rust/Tile assume ("Too many sync wait commands").  Hoist all
    but one semaphore wait per instruction onto nop instructions inserted
    just before it in the same engine's program order (semantically
    identical: all waits still complete before the instruction runs)."""
    from concourse._compat import not_none

    def make_nop(engine, wait):
        nop = nc.engines[engine].nop(nofuse=True)
        nop.ins.sync_info = bass_rust.SyncInfo(on_wait=[wait], on_update=[])
        return nop.ins

    tail_bb = not_none(nc.cur_bb).bb
    plans = []
    for fn in nc.m.functions:
        for bb in fn.blocks:
            plan = {}
            for inst in bb.instructions:
                si = inst.sync_info
                waits = list(si.on_wait) if si and si.on_wait else []
                sem = [w for w in waits if w.sync_type == "semaphore"]
                if len(sem) > 1:
                    plan[inst.name] = sem[:-1]
            if plan:
                plans.append((bb, plan))
    created = {}
    n_tail_before = len(tail_bb.instructions)
    for bb, plan in plans:
        eng_of = {i.name: i.engine for i in bb.instructions}
        for iname, hoists in plan.items():
            created[iname] = [make_nop(eng_of[iname], w) for w in hoists]
    created_names = {n.name for nops in created.values() for n in nops}
    tail_insts = [i for i in tail_bb.instructions if i.name not in created_names]
    assert len(tail_insts) == n_tail_before
    tail_bb.instructions = tail_insts
    for bb, plan in plans:
        out = []
        for inst in bb.instructions:
            if inst.name in plan:
                hoisted = plan[inst.name]
                out.extend(created[inst.name])
                si = inst.sync_info
                si.on_wait = [w for w in si.on_wait if w not in hoisted]
            out.append(inst)
        bb.instructions = out


def build_kernel():
    nc = bass.Bass("TRN2", target_bir_lowering=False, debug=False)

    xT = nc.dram_tensor("xT", [HIDDEN, T], BF16, kind="ExternalInput")
    wq_t = nc.dram_tensor("wq_t", [HIDDEN, HD], BF16, kind="ExternalInput")
    wk_t = nc.dram_tensor("wk_t", [HIDDEN, HD], BF16, kind="ExternalInput")
    wv_t = nc.dram_tensor("wv_t", [HIDDEN, HD], BF16, kind="ExternalInput")
    wo_t = nc.dram_tensor("wo_t", [HD, HIDDEN], BF16, kind="ExternalInput")
    cos2 = nc.dram_tensor("cos2", [P, T], BF16, kind="ExternalInput")
    sin2 = nc.dram_tensor("sin2", [P, T], BF16, kind="ExternalInput")
    yT = nc.dram_tensor("yT", [HIDDEN, T], BF16, kind="ExternalOutput")
    # second/third partial outputs: each output-projection pass stores its
    # partial directly (no on-chip accumulate); the host sums them (it
    # already sums partials across cores).
    yT2 = nc.dram_tensor("yT2", [HIDDEN, T], BF16, kind="ExternalOutput")
    yT3 = nc.dram_tensor("yT3", [HIDDEN, NQ], BF16, kind="ExternalOutput")

    mm = nc.tensor.matmul
    PAIRSWAP = [i ^ 1 for i in range(32)]

    with tile.TileContext(nc) as tc:
        with (
            nc.allow_low_precision(
                reason="bf16 matmul operands (fp32 PSUM accumulation); "
                "resid_var ~4e-5 end to end"
            ),
            tc.tile_pool(name="persist", bufs=1) as persist,
            tc.tile_pool(name="pt_pool", bufs=4) as pt_pool,
            tc.tile_pool(name="nrm_pool", bufs=2) as nrm_pool,
            tc.tile_pool(name="p1", bufs=3) as p1,
            tc.tile_pool(name="p3", bufs=4) as p3,
            # All PSUM comes from one shared pool (4 slots x 2 banks = all 8
            # banks) so phases pipeline without pool-boundary barriers.
            tc.tile_pool(name="psum", bufs=4, space="PSUM") as psum,
            tc.tile_pool(name="dram_pool", bufs=4, space="DRAM") as dram_pool,
        ):
            # ---- persistent SBUF tensors --------------------------------
            # All streaming tensors are split into [P, NQ] half-tiles:
            # dependency tracking is PER TILE, so whole-[P,T] tiles would
            # serialize the interleaved schedule with false WAR/RAW waits.
            qTr = [
                [
                    persist.tile([P, NQ], BF16, tag=f"qTr{m}_{n}", name=f"qTr{m}_{n}")
                    for n in range(2)
                ]
                for m in range(2)
            ]
            kTr = [
                [
                    persist.tile([P, NQ], BF16, tag=f"kTr{m}_{n}", name=f"kTr{m}_{n}")
                    for n in range(2)
                ]
                for m in range(2)
            ]
            v_sb = [
                persist.tile([P, HPC, VW], BF16, tag=f"v{kt}", name=f"v{kt}")
                for kt in range(NKT)
            ]
            otn = [
                [
                    persist.tile([P, NQ], BF16, tag=f"otn{m}_{n}", name=f"otn{m}_{n}")
                    for n in range(2)
                ]
                for m in range(2)
            ]
            wo_sb = persist.tile([P, 2, HIDDEN], BF16, tag="wo_sb", name="wo_sb")
            cos_sb = [
                persist.tile([P, NQ], BF16, tag=f"cos{n}", name=f"cos{n}")
                for n in range(2)
            ]
            sin_sb = [
                persist.tile([P, NQ], BF16, tag=f"sin{n}", name=f"sin{n}")
                for n in range(2)
            ]
            w_sbs = {}
            for name in ("q", "k", "v"):
                w_sbs[name] = persist.tile(
                    [P, IC_CH, HD], BF16, tag=f"w_{name}", name=f"w_{name}"
                )
            x_sb = [
                [
                    persist.tile([P, NQ], BF16, tag=f"x{c}_{t}", name=f"x{c}_{t}")
                    for t in range(2)
                ]
                for c in range(IC_CH)
            ]

            # ---- DMA preloads, priority order on the sync ring ----------
            nc.sync.dma_start(
                out=w_sbs["k"][:], in_=wk_t.rearrange("(c p) o -> p c o", p=P)
            )
            nc.sync.dma_start(
                out=w_sbs["q"][:], in_=wq_t.rearrange("(c p) o -> p c o", p=P)
            )
            for c in range(IC_CH):
                nc.sync.dma_start(
                    out=x_sb[c][0][:], in_=xT[c * P : (c + 1) * P, 0:NQ]
                )
            nc.sync.dma_start(
                out=w_sbs["v"][:], in_=wv_t.rearrange("(c p) o -> p c o", p=P)
            )
            nc.sync.dma_start(out=cos_sb[0][:], in_=cos2[:, 0:NQ])
            nc.sync.dma_start(out=sin_sb[0][:], in_=sin2[:, 0:NQ])
            for c in range(IC_CH):
                nc.sync.dma_start(
                    out=x_sb[c][1][:], in_=xT[c * P : (c + 1) * P, NQ:T]
                )
            nc.sync.dma_start(out=cos_sb[1][:], in_=cos2[:, NQ:T])
            nc.sync.dma_start(out=sin_sb[1][:], in_=sin2[:, NQ:T])
            nc.sync.dma_start(
                out=wo_sb[:], in_=wo_t.rearrange("(c p) o -> p c o", p=P)
            )
            for kt in range(NKT):
                nc.vector.memset(
                    v_sb[kt][:, :, D : D + 1].bitcast(mybir.dt.uint16),
                    0x3F80,  # bf16 bits of 1.0
                )
            # warm the ACT engine's Exp/Ln table during the DMA preload so
            # the first real exp doesn't pay the ACT_TABLE_LOAD
            warm_in = p1.tile([1, 64], F32, tag="warm_in", name="warm_in")
            warm_out = p1.tile([1, 64], BF16, tag="warm_out", name="warm_out")
            nc.vector.memset(warm_in[:], 0.0)
            nc.scalar.activation(
                out=warm_out[:],
                in_=warm_in[:],
                func=mybir.ActivationFunctionType.Exp,
            )

            # ---- q/k projection of one [128, NQ] tile + RoPE -------------
            def proj_qk_tile(name, m, n):
                """Generator: 8 c-chunk matmuls (yield per 2) + RoPE."""
                w_sb = w_sbs[name]
                dst = (qTr if name == "q" else kTr)[m][n]
                ps = psum.tile([P, NQ], F32, tag="main", name="ps")
                for sub in range(2):
                    ss = slice(sub * 512, (sub + 1) * 512)
                    for c in range(IC_CH):
                        mm(
                            ps[:, ss],
                            w_sb[:, c, m * P : (m + 1) * P],
                            x_sb[c][n][:, ss],
                            start=(c == 0),
                            stop=(c == IC_CH - 1),
                        )
                        if c % 4 == 3 and not (sub == 1 and c == IC_CH - 1):
                            yield
                # RoPE: dst = q*cos + pairswap(q*sin_host) where sin_host is
                # the sign-baked sin table pre-pairswapped on host, so the
                # shuffle runs SBUF->SBUF (PSUM input is not supported).
                tmp = p1.tile([P, NQ], BF16, tag="tmp", name="tmp")
                nc.vector.tensor_mul(out=tmp[:], in0=ps[:], in1=sin_sb[n][:])
                rot = p1.tile([P, NQ], BF16, tag="rot", name="rot")
                nc.vector.stream_shuffle(out=rot[:], in_=tmp[:], mask=PAIRSWAP)
                nc.vector.tensor_mul(out=dst[:], in0=ps[:], in1=cos_sb[n][:])
                nc.gpsimd.tensor_add(out=dst[:], in0=dst[:], in1=rot[:])
                yield

            # ---- v projection, one k-tile per unit -----------------------
            def proj_v_units():
                for kt in range(NKT):
                    t, tc_ = divmod(kt, 8)
                    psv = psum.tile([P, HPC, D], F32, tag="main", name="psv")
                    for c in range(IC_CH):
                        mm(
                            psv[:, :, :],
                            x_sb[c][t][:, tc_ * P : (tc_ + 1) * P],
                            w_sbs["v"][:, c, :],
                            start=(c == 0),
                            stop=(c == IC_CH - 1),
                        )
                    nc.vector.tensor_copy(
                        out=v_sb[kt][:, :, 0:D], in_=psv[:, :, :]
                    )
                    yield

            # ---- output projection, pass-split over head-pairs -----------
            def outproj_units(hf, cpass, rows=(0, P), dst=None, dst_hf=None,
                              act_evict=False):
                """One unit per 128-row output chunk mo: 2 sub-matmuls over
                head-pair `cpass` (contraction rows `rows`), eviction to bf16
                and a store into the partial output `dst`.  act_evict
                alternates evictions onto the ACT engine (only safe when the
                exp stream is done)."""
                if dst is None:
                    dst = yT
                dcols = slice(dst_hf * NQ, (dst_hf + 1) * NQ) if dst_hf is not None \
                    else slice(0, NQ)
                for mo in range(HIDDEN // P):
                    ps = psum.tile([P, NQ], F32, tag="main", name="psy")
                    for sub in range(2):
                        ss = slice(sub * 512, (sub + 1) * 512)
                        mm(
                            ps[:, ss],
                            wo_sb[rows[0] : rows[1], cpass, mo * P : (mo + 1) * P],
                            otn[cpass][hf][rows[0] : rows[1], ss],
                            start=True,
                            stop=True,
                        )
                    ysb = p3.tile([P, NQ], BF16, tag="ysb", name="ysb")
                    if act_evict and mo % 2 == 1:
                        nc.scalar.copy(out=ysb[:], in_=ps[:])
                    else:
                        nc.vector.tensor_copy(out=ysb[:], in_=ps[:])
                    nc.gpsimd.dma_start(
                        out=dst[mo * P : (mo + 1) * P, dcols], in_=ysb[:]
                    )
                    yield

            # ---- attention for one (head, q-half) ------------------------
            LAG = 2  # attnv trails exp by 2 k-tiles so it never waits on ACT
            FIN_SLOT = 4  # kt at which the previous attend's norm finishes

            def attend(h, hf, pulls, master, finish_prev=None):
                m = h // 2
                r0 = (h % 2) * D
                # [P, NQ] = 2 PSUM banks: rows 0..63 = O accumulator, row 64
                # = exp-sum (ones column), rows 64..127 reused afterwards for
                # the broadcast reciprocal (no extra PSUM slot needed).
                ot = psum.tile([P, NQ], F32, tag="main", name="ot")
                pts = {}

                def attnv(j):
                    pt = pts.pop(j)
                    for sub in range(2):
                        ss = slice(sub * 512, (sub + 1) * 512)
                        mm(
                            ot[0:VW, ss],
                            v_sb[j][:, h, :],
                            pt[:, ss],
                            start=(j == 0),
                            stop=(j == NKT - 1),
                        )

                for kt in range(NKT):
                    kn, kc = divmod(kt, 8)
                    pt = pts[kt] = pt_pool.tile([P, NQ], BF16, tag="pt", name="pt")
                    st = psum.tile([P, NQ], F32, tag="main", name="st")
                    for sub in range(2):
                        q0 = sub * 512
                        mm(
                            st[:, sub * 512 : (sub + 1) * 512],
                            kTr[m][kn][r0 : r0 + D, kc * P : (kc + 1) * P],
                            qTr[m][hf][r0 : r0 + D, q0 : q0 + 512],
                            start=True,
                            stop=True,
                        )
                    if kt == FIN_SLOT and finish_prev is not None:
                        finish_prev()
                    for _ in range(pulls[kt] if kt < len(pulls) else 0):
                        next(master, None)
                    nc.scalar.activation(
                        out=pt[:],
                        in_=st[:],
                        func=mybir.ActivationFunctionType.Exp,
                        scale=float(1.0 / np.sqrt(D)),
                    )
                    if kt >= LAG:
                        attnv(kt - LAG)
                for j in range(NKT - LAG, NKT):
                    for _ in range(pulls[j + LAG] if j + LAG < len(pulls) else 0):
                        next(master, None)
                    attnv(j)
                # softmax denominator reciprocal without any table switch or
                # DRAM bounce: 1/den = exp(-ln(den)), both funcs live in the
                # same ACT table as the score exp.  The broadcast + multiply
                # are deferred into the next attend (finish()).
                ln_row = nrm_pool.tile([1, NQ], F32, tag="lnr", name="lnr")
                nc.scalar.activation(
                    out=ln_row[:],
                    in_=ot[D : D + 1, :],
                    func=mybir.ActivationFunctionType.Ln,
                )
                inv_bf = nrm_pool.tile([1, NQ], BF16, tag="invd", name="invd")
                nc.scalar.activation(
                    out=inv_bf[:],
                    in_=ln_row[:],
                    func=mybir.ActivationFunctionType.Exp,
                    scale=-1.0,
                )
                # broadcast 1/den across partitions via a DRAM bounce on the
                # sync ring (stride-0 partition reads need a DRAM source);
                # both DMAs run during the attend boundary, off every engine.
                dinv = dram_pool.tile([1, NQ], BF16, tag="dinv", name="dinv")
                nc.sync.dma_start(out=dinv[:], in_=inv_bf[:])
                rb = nrm_pool.tile([D, NQ], BF16, tag="rb", name="rb")
                src = dinv[0:1, :]
                nc.sync.dma_start(
                    out=rb[:],
                    in_=bass.AP(
                        tensor=src.tensor,
                        offset=src.offset,
                        ap=[[0, D]] + [list(a) for a in src.ap[1:]],
                    ),
                )

                def finish():
                    nc.vector.tensor_mul(
                        out=otn[m][hf][r0 : r0 + D, :],
                        in0=ot[0:D, :],
                        in1=rb[:],
                    )

                return finish

            # ---- emission schedule --------------------------------------
            # upfront: k/q projections (m=0, n=0); v starts as attend(0,0)
            # fillers (wv arrives after the x t0 pieces).
            for _ in proj_qk_tile("k", 0, 0):
                pass
            for _ in proj_qk_tile("q", 0, 0):
                pass

            gens = {
                "V": proj_v_units(),
                "K01": proj_qk_tile("k", 0, 1),
                "Q01": proj_qk_tile("q", 0, 1),
                "K10": proj_qk_tile("k", 1, 0),
                "Q10": proj_qk_tile("q", 1, 0),
                "K11": proj_qk_tile("k", 1, 1),
                "Q11": proj_qk_tile("q", 1, 1),
                "A0": outproj_units(0, 0, dst=yT2, dst_hf=0),
                "A1": outproj_units(1, 0, dst=yT2, dst_hf=1),
                "B0": outproj_units(0, 1, dst=yT, dst_hf=0),
                # q-half-1 pass B split by head so its first half fills the
                # last attend; head 3 (B2b below) is the only true tail.
                "B1a": outproj_units(1, 1, rows=(0, D), dst=yT, dst_hf=1),
            }
            master_order = (
                ["V"] * 4                                  # A1: 20
                + ["K01"] * 2 + ["V"] + ["K01"] * 2 + ["V"] * 11
                + ["Q01"] * 4 + ["K10"] * 2                # A2: 6
                + ["K10"] * 2 + ["Q10"] * 4                # A3: 6
                + ["A0"] * 2 + ["K11"] * 4 + ["A0"] * 2    # A4: 8
                + ["A0"] * 4 + ["A1"] * 2                  # A5: 6
                + ["Q11"] * 4 + ["A1"] * 3                 # A6: 7
                + ["A1"] * 3 + ["B0"] * 3                  # A7: 6
                + ["B0"] * 1 + ["B1a"] * 8                 # A8: 9
            )
            # B0's remaining 4 units are deliberately NOT pulled: the safety
            # drain below emits them right after the last attend, keeping the
            # PE warm through the final normalization chain.

            def master():
                for sym in master_order:
                    yield next(gens[sym], None)

            ms = master()
            # filler pulls per kt slot (18 slots: 16 kts + 2 attnv-tail
            # slots).  After each attend boundary, pulls only start at kt>=3:
            # until the previous attend's deferred norm (FIN_SLOT) frees its
            # ot PSUM slot, all four psum slots are live.
            SP = {
                0: [2, 2, 2, 2, 2, 2, 1, 1, 1, 1, 1, 1, 1, 1, 0, 0, 0, 0],  # 20
                1: [0, 0, 0, 0, 0, 0, 1, 1, 0, 1, 0, 1, 0, 1, 0, 0, 1, 0],  # 6
                2: [0, 0, 0, 0, 0, 0, 1, 1, 0, 1, 0, 1, 0, 1, 0, 0, 1, 0],  # 6
                3: [0, 0, 0, 0, 0, 0, 1, 1, 1, 1, 1, 1, 0, 1, 0, 0, 1, 0],  # 8
                4: [0, 0, 0, 0, 0, 0, 1, 1, 0, 1, 0, 1, 0, 1, 0, 0, 1, 0],  # 6
                5: [0, 0, 0, 0, 0, 0, 1, 1, 1, 1, 0, 1, 0, 1, 0, 0, 1, 0],  # 7
                6: [0, 0, 0, 0, 0, 0, 1, 1, 0, 1, 0, 1, 0, 1, 0, 0, 1, 0],  # 6
                7: [0, 0, 0, 0, 0, 0, 1, 1, 1, 1, 1, 1, 1, 1, 0, 0, 1, 0],  # 9
            }
            order = [(0, 0), (1, 0), (0, 1), (1, 1), (2, 0), (3, 0), (2, 1), (3, 1)]
            fin = None
            for i, (h, hf) in enumerate(order):
                fin = attend(h, hf, SP[i], ms, finish_prev=fin)
            for _ in ms:  # drain any master remainder
                pass
            # B0's reserved store units keep the PE warm while the last
            # attend's norm completes on the ACT engine.
            for g in gens.values():
                for _ in g:
                    pass
            fin()  # normalize the last attend
            # tail: q-half-1 pass B, head 3 rows only, into its own partial
            for _ in outproj_units(1, 1, rows=(D, P), dst=yT3, act_evict=True):
                pass
    _split_waits(nc)
    return nc


_PERM = None


def _perm64():
    global _PERM
    if _PERM is None:
        p = np.empty(D, dtype=np.int64)
        p[0::2] = np.arange(32)
        p[1::2] = np.arange(32) + 32
        _PERM = p
    return _PERM


def _rope_tables():
    perm = _perm64()
    inv_freq = 1.0 / (10000.0 ** (np.arange(0, D, 2, dtype=np.float32) / D))
    t = np.arange(T, dtype=np.float32)
    freqs = t[:, None] * inv_freq[None, :]  # [T, 32]
    emb = np.concatenate((freqs, freqs), axis=-1)  # [T, 64]
    cos = np.cos(emb).T.astype(np.float32)[perm]  # [64, T], permuted
    sin = np.sin(emb).T.astype(np.float32)[perm]
    sign = np.where(np.arange(D) % 2 == 0, -1.0, 1.0).astype(np.float32)
    sin_signed = sin * sign[:, None]
    # pre-pairswap: kernel computes rot = pairswap(q * sin_host), wants
    # rot[p] = q[p^1] * sin_signed[p], so sin_host[p] = sin_signed[p^1]
    sin_host = sin_signed[[p ^ 1 for p in range(D)]]
    cos2 = np.ascontiguousarray(np.concatenate([cos, cos], axis=0))  # [128,T]
    sin2 = np.ascontiguousarray(np.concatenate([sin_host, sin_host], 0))
    return cos2, sin2


def make_in_maps(x, wq, wk, wv, wo):
    import ml_dtypes

    bf = ml_dtypes.bfloat16
    perm = _perm64()
    cos2, sin2 = _rope_tables()

    def permute_heads(w):  # [256, 1024] -> row-permuted within each head
        return w.reshape(HPC, D, HIDDEN)[:, perm, :].reshape(HD, HIDDEN)

    in_maps = []
    for core in range(N_CORES):
        b, g = divmod(core, N_CORES // B)
        hs = slice(g * HD, (g + 1) * HD)
        in_maps.append(
            {
                "xT": np.ascontiguousarray(x[b].T).astype(bf),
                "wq_t": np.ascontiguousarray(permute_heads(wq[hs]).T).astype(bf),
                "wk_t": np.ascontiguousarray(permute_heads(wk[hs]).T).astype(bf),
                "wv_t": np.ascontiguousarray(wv[hs].T).astype(bf),
                "wo_t": np.ascontiguousarray(wo[:, hs].T).astype(bf),
                "cos2": cos2.astype(bf),
                "sin2": sin2.astype(bf),
            }
        )
    return in_maps


def gather_output(results):
    y = np.zeros((B, T, HIDDEN), dtype=np.float32)
    for core, res in enumerate(results):
        b = core // (N_CORES // B)
        y[b] += res["yT"].T.astype(np.float32)
        y[b] += res["yT2"].T.astype(np.float32)
        y[b, NQ:T, :] += res["yT3"].T.astype(np.float32)
    return y


_NC = None


def kernel(x, wq, wk, wv, wo):
    global _NC
    import time

    from concourse.bass_utils import run_bass_kernel_spmd

    if _NC is None:
        _NC = build_kernel()
    in_maps = make_in_maps(
        np.asarray(x), np.asarray(wq), np.asarray(wk), np.asarray(wv), np.asarray(wo)
    )
    try:
        res = run_bass_kernel_spmd(_NC, in_maps, core_ids=list(range(N_CORES)))
    except Exception:
        # transient device wedge (e.g. NRT_EXEC_UNIT_UNRECOVERABLE from a
        # prior run) -- retry once
        time.sleep(2.0)
        res = run_bass_kernel_spmd(_NC, in_maps, core_ids=list(range(N_CORES)))
    return gather_output(res.results)


# revision 43
# speedup vs baseline: 1.0421x; 1.0421x over previous
"""Self-contained Trainium2 Bass kernel for nn_Attention_41472204210330.

Multi-head attention (B=2, T=2048, HIDDEN=1024, 16 heads, head_dim=64, fp32)
with RoPE, sharded over 8 NeuronCores: data-parallel over the batch (2) x
tensor-parallel over heads (4 groups of 4 heads).  Each core computes its
batch's q/k/v projections for its 4 heads, RoPE, attention, and a partial
output projection (its heads' slice of wo); the host sums the 4 partials per
batch element.

Performance design (v2): the PE p-state/HAM clock only stays at full speed
(2.4 GHz) while the PE instruction stream is gap-free, so the whole kernel is
emitted as ONE dense PE schedule:
  - xT is DMA'd in 16 [128,1024] pieces in priority order so the first
    projection matmul issues ~1.6us in; everything else preloads behind it.
  - RoPE: the head-dim axis is interleaved host-side ([d0,d32,d1,d33,...]) so
    rotate-half becomes an intra-32 partition pair swap == one DVE
    stream_shuffle (no SBUF-SBUF DMA bounce).  cos/sin tables are bf16 with
    the sign pre-baked.
  - attends are ACT(exp)-bound by ~25%; a master filler sequence (v tiles,
    m=1 q/k projections, and the output projection split into per-head-pair
    accumulation passes A/B) drips PE work into every attend window.
  - softmax denominator via a ones-column in V; normalization is
    reciprocal([1,T] direct from PSUM) + gpsimd partition_broadcast (no DRAM
    bounce).
  - yT partials are stored in bf16 (host sums in fp32).
"""

import sys

if "/opt/trn_rl_repo" not in sys.path:
    sys.path.insert(0, "/opt/trn_rl_repo")

import numpy as np

import bass_rust
import concourse.bass as bass
import concourse.mybir as mybir
import concourse.tile as tile

HIDDEN = 1024
NUM_HEADS = 16
D = 64  # head dim
B = 2
T = 2048
N_CORES = 8
HPC = NUM_HEADS // (N_CORES // B)  # heads per core = 4
HD = HPC * D  # per-core head dims = 256
P = 128
F32 = mybir.dt.float32
BF16 = mybir.dt.bfloat16
NQ = 1024  # q-half width
IC_CH = HIDDEN // P  # 8 input-channel chunks
NKT = T // P  # 16 k tiles
VW = D + 1  # v columns per head incl. ones column


def _split_waits(nc):
    """The in-container walrus caps semaphore waits per instruction lower
    than bass_rust/Tile assume ("Too many sync wait commands").  Hoist all
    but one semaphore wait per instruction onto nop instructions inserted
    just before it in the same engine's program order (semantically
    identical: all waits still complete before the instruction runs)."""
    from concourse._compat import not_none

    def make_nop(engine, wait):
        nop = nc.engines[engine].nop(nofuse=True)
        nop.ins.sync_info = bass_rust.SyncInfo(on_wait=[wait], on_update=[])
        return nop.ins

    tail_bb = not_none(nc.cur_bb).bb
    plans = []
    for fn in nc.m.functions:
        for bb in fn.blocks:
            plan = {}
            for inst in bb.instructions:
                si = inst.sync_info
                waits = list(si.on_wait) if si and si.on_wait else []
                sem = [w for w in waits if w.sync_type == "semaphore"]
                if len(sem) > 1:
                    plan[inst.name] = sem[:-1]
            if plan:
                plans.append((bb, plan))
    created = {}
    n_tail_before = len(tail_bb.instructions)
    for bb, plan in plans:
        eng_of = {i.name: i.engine for i in bb.instructions}
        for iname, hoists in plan.items():
            created[iname] = [make_nop(eng_of[iname], w) for w in hoists]
    created_names = {n.name for nops in created.values() for n in nops}
    tail_insts = [i for i in tail_bb.instructions if i.name not in created_names]
    assert len(tail_insts) == n_tail_before
    tail_bb.instructions = tail_insts
    for bb, plan in plans:
        out = []
        for inst in bb.instructions:
            if inst.name in plan:
                hoisted = plan[inst.name]
                out.extend(created[inst.name])
                si = inst.sync_info
                si.on_wait = [w for w in si.on_wait if w not in hoisted]
            out.append(inst)
        bb.instructions = out


def build_kernel():
    nc = bass.Bass("TRN2", target_bir_lowering=False, debug=False)

    xT = nc.dram_tensor("xT", [HIDDEN, T], BF16, kind="ExternalInput")
    wq_t = nc.dram_tensor("wq_t", [HIDDEN, HD], BF16, kind="ExternalInput")
    wk_t = nc.dram_tensor("wk_t", [HIDDEN, HD], BF16, kind="ExternalInput")
    wv_t = nc.dram_tensor("wv_t", [HIDDEN, HD], BF16, kind="ExternalInput")
    wo_t = nc.dram_tensor("wo_t", [HD, HIDDEN], BF16, kind="ExternalInput")
    cos2 = nc.dram_tensor("cos2", [P, T], BF16, kind="ExternalInput")
    sin2 = nc.dram_tensor("sin2", [P, T], BF16, kind="ExternalInput")
    yT = nc.dram_tensor("yT", [HIDDEN, T], BF16, kind="ExternalOutput")

    mm = nc.tensor.matmul
    PAIRSWAP = [i ^ 1 for i in range(32)]

    with tile.TileContext(nc) as tc:
        with (
            nc.allow_low_precision(
                reason="bf16 matmul operands (fp32 PSUM accumulation); "
                "resid_var ~4e-5 end to end"
            ),
            tc.tile_pool(name="persist", bufs=1) as persist,
            tc.tile_pool(name="pt_pool", bufs=4) as pt_pool,
            tc.tile_pool(name="nrm_pool", bufs=2) as nrm_pool,
            tc.tile_pool(name="p1", bufs=3) as p1,
            tc.tile_pool(name="p3", bufs=4) as p3,
            # All PSUM comes from one shared pool (4 slots x 2 banks = all 8
            # banks) so phases pipeline without pool-boundary barriers.
            tc.tile_pool(name="psum", bufs=4, space="PSUM") as psum,
            tc.tile_pool(name="dram_pool", bufs=4, space="DRAM") as dram_pool,
        ):
            # ---- persistent SBUF tensors --------------------------------
            # All streaming tensors are split into [P, NQ] half-tiles:
            # dependency tracking is PER TILE, so whole-[P,T] tiles would
            # serialize the interleaved schedule with false WAR/RAW waits.
            qTr = [
                [
                    persist.tile([P, NQ], BF16, tag=f"qTr{m}_{n}", name=f"qTr{m}_{n}")
                    for n in range(2)
                ]
                for m in range(2)
            ]
            kTr = [
                [
                    persist.tile([P, NQ], BF16, tag=f"kTr{m}_{n}", name=f"kTr{m}_{n}")
                    for n in range(2)
                ]
                for m in range(2)
            ]
            v_sb = [
                persist.tile([P, HPC, VW], BF16, tag=f"v{kt}", name=f"v{kt}")
                for kt in range(NKT)
            ]
            otn = [
                [
                    persist.tile([P, NQ], BF16, tag=f"otn{m}_{n}", name=f"otn{m}_{n}")
                    for n in range(2)
                ]
                for m in range(2)
            ]
            # passA output-projection partials (bf16), one per q-half
            yA = [
                persist.tile([P, 8, NQ], BF16, tag=f"yA{n}", name=f"yA{n}")
                for n in range(2)
            ]
            wo_sb = persist.tile([P, 2, HIDDEN], BF16, tag="wo_sb", name="wo_sb")
            cos_sb = [
                persist.tile([P, NQ], BF16, tag=f"cos{n}", name=f"cos{n}")
                for n in range(2)
            ]
            sin_sb = [
                persist.tile([P, NQ], BF16, tag=f"sin{n}", name=f"sin{n}")
                for n in range(2)
            ]
            w_sbs = {}
            for name in ("q", "k", "v"):
                w_sbs[name] = persist.tile(
                    [P, IC_CH, HD], BF16, tag=f"w_{name}", name=f"w_{name}"
                )
            x_sb = [
                [
                    persist.tile([P, NQ], BF16, tag=f"x{c}_{t}", name=f"x{c}_{t}")
                    for t in range(2)
                ]
                for c in range(IC_CH)
            ]

            # ---- DMA preloads, priority order on the sync ring ----------
            nc.sync.dma_start(
                out=w_sbs["k"][:], in_=wk_t.rearrange("(c p) o -> p c o", p=P)
            )
            nc.sync.dma_start(
                out=w_sbs["q"][:], in_=wq_t.rearrange("(c p) o -> p c o", p=P)
            )
            for c in range(IC_CH):
                nc.sync.dma_start(
                    out=x_sb[c][0][:], in_=xT[c * P : (c + 1) * P, 0:NQ]
                )
            nc.sync.dma_start(
                out=w_sbs["v"][:], in_=wv_t.rearrange("(c p) o -> p c o", p=P)
            )
            nc.sync.dma_start(out=cos_sb[0][:], in_=cos2[:, 0:NQ])
            nc.sync.dma_start(out=sin_sb[0][:], in_=sin2[:, 0:NQ])
            for c in range(IC_CH):
                nc.sync.dma_start(
                    out=x_sb[c][1][:], in_=xT[c * P : (c + 1) * P, NQ:T]
                )
            nc.sync.dma_start(out=cos_sb[1][:], in_=cos2[:, NQ:T])
            nc.sync.dma_start(out=sin_sb[1][:], in_=sin2[:, NQ:T])
            nc.sync.dma_start(
                out=wo_sb[:], in_=wo_t.rearrange("(c p) o -> p c o", p=P)
            )
            for kt in range(NKT):
                nc.vector.memset(
                    v_sb[kt][:, :, D : D + 1].bitcast(mybir.dt.uint16),
                    0x3F80,  # bf16 bits of 1.0
                )
            # warm the ACT engine's Exp/Ln table during the DMA preload so
            # the first real exp doesn't pay the ACT_TABLE_LOAD
            warm_in = p1.tile([1, 64], F32, tag="warm_in", name="warm_in")
            warm_out = p1.tile([1, 64], BF16, tag="warm_out", name="warm_out")
            nc.vector.memset(warm_in[:], 0.0)
            nc.scalar.activation(
                out=warm_out[:],
                in_=warm_in[:],
                func=mybir.ActivationFunctionType.Exp,
            )

            # ---- q/k projection of one [128, NQ] tile + RoPE -------------
            def proj_qk_tile(name, m, n):
                """Generator: 8 c-chunk matmuls (yield per 2) + RoPE."""
                w_sb = w_sbs[name]
                dst = (qTr if name == "q" else kTr)[m][n]
                ps = psum.tile([P, NQ], F32, tag="main", name="ps")
                for sub in range(2):
                    ss = slice(sub * 512, (sub + 1) * 512)
                    for c in range(IC_CH):
                        mm(
                            ps[:, ss],
                            w_sb[:, c, m * P : (m + 1) * P],
                            x_sb[c][n][:, ss],
                            start=(c == 0),
                            stop=(c == IC_CH - 1),
                        )
                        if c % 4 == 3 and not (sub == 1 and c == IC_CH - 1):
                            yield
                # RoPE: dst = q*cos + pairswap(q*sin_host) where sin_host is
                # the sign-baked sin table pre-pairswapped on host, so the
                # shuffle runs SBUF->SBUF (PSUM input is not supported).
                tmp = p1.tile([P, NQ], BF16, tag="tmp", name="tmp")
                nc.vector.tensor_mul(out=tmp[:], in0=ps[:], in1=sin_sb[n][:])
                rot = p1.tile([P, NQ], BF16, tag="rot", name="rot")
                nc.vector.stream_shuffle(out=rot[:], in_=tmp[:], mask=PAIRSWAP)
                nc.vector.tensor_mul(out=dst[:], in0=ps[:], in1=cos_sb[n][:])
                nc.gpsimd.tensor_add(out=dst[:], in0=dst[:], in1=rot[:])
                yield

            # ---- v projection, one k-tile per unit -----------------------
            def proj_v_units():
                for kt in range(NKT):
                    t, tc_ = divmod(kt, 8)
                    psv = psum.tile([P, HPC, D], F32, tag="main", name="psv")
                    for c in range(IC_CH):
                        mm(
                            psv[:, :, :],
                            x_sb[c][t][:, tc_ * P : (tc_ + 1) * P],
                            w_sbs["v"][:, c, :],
                            start=(c == 0),
                            stop=(c == IC_CH - 1),
                        )
                    nc.vector.tensor_copy(
                        out=v_sb[kt][:, :, 0:D], in_=psv[:, :, :]
                    )
                    yield

            # ---- output projection, pass-split over head-pairs -----------
            def outproj_units(hf, cpass, rows=(0, P), mode=None):
                """One unit per 128-row output chunk mo: 2 sub-matmuls over
                head-pair `cpass` (contraction rows `rows`) + eviction.
                mode: 'toA' -> copy into yA; 'accA' -> add into yA in place;
                'store' -> add yA + store to yT."""
                if mode is None:
                    mode = "toA" if cpass == 0 else "store"
                for mo in range(HIDDEN // P):
                    ps = psum.tile([P, NQ], F32, tag="main", name="psy")
                    for sub in range(2):
                        ss = slice(sub * 512, (sub + 1) * 512)
                        mm(
                            ps[:, ss],
                            wo_sb[rows[0] : rows[1], cpass, mo * P : (mo + 1) * P],
                            otn[cpass][hf][rows[0] : rows[1], ss],
                            start=True,
                            stop=True,
                        )
                    if mode == "toA":
                        nc.vector.tensor_copy(out=yA[hf][:, mo, :], in_=ps[:])
                    elif mode == "accA":
                        nc.vector.tensor_add(
                            out=yA[hf][:, mo, :], in0=ps[:], in1=yA[hf][:, mo, :]
                        )
                    else:
                        ysb = p3.tile([P, NQ], BF16, tag="ysb", name="ysb")
                        nc.vector.tensor_add(
                            out=ysb[:], in0=ps[:], in1=yA[hf][:, mo, :]
                        )
                        nc.sync.dma_start(
                            out=yT[mo * P : (mo + 1) * P, hf * NQ : (hf + 1) * NQ],
                            in_=ysb[:],
                        )
                    yield

            # ---- attention for one (head, q-half) ------------------------
            LAG = 2  # attnv trails exp by 2 k-tiles so it never waits on ACT
            FIN_SLOT = 4  # kt at which the previous attend's norm finishes

            def attend(h, hf, pulls, master, finish_prev=None):
                m = h // 2
                r0 = (h % 2) * D
                # [P, NQ] = 2 PSUM banks: rows 0..63 = O accumulator, row 64
                # = exp-sum (ones column), rows 64..127 reused afterwards for
                # the broadcast reciprocal (no extra PSUM slot needed).
                ot = psum.tile([P, NQ], F32, tag="main", name="ot")
                pts = {}

                def attnv(j):
                    pt = pts.pop(j)
                    for sub in range(2):
                        ss = slice(sub * 512, (sub + 1) * 512)
                        mm(
                            ot[0:VW, ss],
                            v_sb[j][:, h, :],
                            pt[:, ss],
                            start=(j == 0),
                            stop=(j == NKT - 1),
                        )

                for kt in range(NKT):
                    kn, kc = divmod(kt, 8)
                    pt = pts[kt] = pt_pool.tile([P, NQ], BF16, tag="pt", name="pt")
                    st = psum.tile([P, NQ], F32, tag="main", name="st")
                    for sub in range(2):
                        q0 = sub * 512
                        mm(
                            st[:, sub * 512 : (sub + 1) * 512],
                            kTr[m][kn][r0 : r0 + D, kc * P : (kc + 1) * P],
                            qTr[m][hf][r0 : r0 + D, q0 : q0 + 512],
                            start=True,
                            stop=True,
                        )
                    if kt == FIN_SLOT and finish_prev is not None:
                        finish_prev()
                    for _ in range(pulls[kt] if kt < len(pulls) else 0):
                        next(master, None)
                    nc.scalar.activation(
                        out=pt[:],
                        in_=st[:],
                        func=mybir.ActivationFunctionType.Exp,
                        scale=float(1.0 / np.sqrt(D)),
                    )
                    if kt >= LAG:
                        attnv(kt - LAG)
                for j in range(NKT - LAG, NKT):
                    for _ in range(pulls[j + LAG] if j + LAG < len(pulls) else 0):
                        next(master, None)
                    attnv(j)
                # softmax denominator reciprocal without any table switch or
                # DRAM bounce: 1/den = exp(-ln(den)), both funcs live in the
                # same ACT table as the score exp.  The broadcast + multiply
                # are deferred into the next attend (finish()).
                ln_row = nrm_pool.tile([1, NQ], F32, tag="lnr", name="lnr")
                nc.scalar.activation(
                    out=ln_row[:],
                    in_=ot[D : D + 1, :],
                    func=mybir.ActivationFunctionType.Ln,
                )
                inv_bf = nrm_pool.tile([1, NQ], BF16, tag="invd", name="invd")
                nc.scalar.activation(
                    out=inv_bf[:],
                    in_=ln_row[:],
                    func=mybir.ActivationFunctionType.Exp,
                    scale=-1.0,
                )
                # broadcast 1/den across partitions via a DRAM bounce on the
                # sync ring (stride-0 partition reads need a DRAM source);
                # both DMAs run during the attend boundary, off every engine.
                dinv = dram_pool.tile([1, NQ], BF16, tag="dinv", name="dinv")
                nc.sync.dma_start(out=dinv[:], in_=inv_bf[:])
                rb = nrm_pool.tile([D, NQ], BF16, tag="rb", name="rb")
                src = dinv[0:1, :]
                nc.sync.dma_start(
                    out=rb[:],
                    in_=bass.AP(
                        tensor=src.tensor,
                        offset=src.offset,
                        ap=[[0, D]] + [list(a) for a in src.ap[1:]],
                    ),
                )

                def finish():
                    nc.vector.tensor_mul(
                        out=otn[m][hf][r0 : r0 + D, :],
                        in0=ot[0:D, :],
                        in1=rb[:],
                    )

                return finish

            # ---- emission schedule --------------------------------------
            # upfront: k/q projections (m=0, n=0); v starts as attend(0,0)
            # fillers (wv arrives after the x t0 pieces).
            for _ in proj_qk_tile("k", 0, 0):
                pass
            for _ in proj_qk_tile("q", 0, 0):
                pass

            gens = {
                "V": proj_v_units(),
                "K01": proj_qk_tile("k", 0, 1),
                "Q01": proj_qk_tile("q", 0, 1),
                "K10": proj_qk_tile("k", 1, 0),
                "Q10": proj_qk_tile("q", 1, 0),
                "K11": proj_qk_tile("k", 1, 1),
                "Q11": proj_qk_tile("q", 1, 1),
                "A0": outproj_units(0, 0),
                "A1": outproj_units(1, 0),
                "B0": outproj_units(0, 1),
                # q-half-1 pass B split by head so its first half fills the
                # last attend: head 2 accumulates into yA in place, head 3
                # (B2b below) is the only true tail.
                "B1a": outproj_units(1, 1, rows=(0, D), mode="accA"),
            }
            master_order = (
                ["V"] * 4                                  # A1: 20
                + ["K01"] * 2 + ["V"] + ["K01"] * 2 + ["V"] * 11
                + ["Q01"] * 4 + ["K10"] * 2                # A2: 6
                + ["K10"] * 2 + ["Q10"] * 4                # A3: 6
                + ["A0"] * 2 + ["K11"] * 4 + ["A0"] * 2    # A4: 8
                + ["A0"] * 4 + ["A1"] * 2                  # A5: 6
                + ["Q11"] * 4 + ["A1"] * 3                 # A6: 7
                + ["A1"] * 3 + ["B0"] * 3                  # A7: 6
                + ["B0"] * 1 + ["B1a"] * 8                 # A8: 9
            )
            # B0's remaining 4 units are deliberately NOT pulled: the safety
            # drain below emits them right after the last attend, keeping the
            # PE warm through the final normalization chain.

            def master():
                for sym in master_order:
                    yield next(gens[sym], None)

            ms = master()
            # filler pulls per kt slot (18 slots: 16 kts + 2 attnv-tail
            # slots).  After each attend boundary, pulls only start at kt>=3:
            # until the previous attend's deferred norm (FIN_SLOT) frees its
            # ot PSUM slot, all four psum slots are live.
            SP = {
                0: [2, 2, 2, 2, 2, 2, 1, 1, 1, 1, 1, 1, 1, 1, 0, 0, 0, 0],  # 20
                1: [0, 0, 0, 0, 0, 0, 1, 1, 0, 1, 0, 1, 0, 1, 0, 0, 1, 0],  # 6
                2: [0, 0, 0, 0, 0, 0, 1, 1, 0, 1, 0, 1, 0, 1, 0, 0, 1, 0],  # 6
                3: [0, 0, 0, 0, 0, 0, 1, 1, 1, 1, 1, 1, 0, 1, 0, 0, 1, 0],  # 8
                4: [0, 0, 0, 0, 0, 0, 1, 1, 0, 1, 0, 1, 0, 1, 0, 0, 1, 0],  # 6
                5: [0, 0, 0, 0, 0, 0, 1, 1, 1, 1, 0, 1, 0, 1, 0, 0, 1, 0],  # 7
                6: [0, 0, 0, 0, 0, 0, 1, 1, 0, 1, 0, 1, 0, 1, 0, 0, 1, 0],  # 6
                7: [0, 0, 0, 0, 0, 0, 1, 1, 1, 1, 1, 1, 1, 1, 0, 0, 1, 0],  # 9
            }
            order = [(0, 0), (1, 0), (0, 1), (1, 1), (2, 0), (3, 0), (2, 1), (3, 1)]
            fin = None
            for i, (h, hf) in enumerate(order):
                fin = attend(h, hf, SP[i], ms, finish_prev=fin)
            for _ in ms:  # drain any master remainder
                pass
            # B0's reserved store units keep the PE warm while the last
            # attend's norm completes on the ACT engine.
            for g in gens.values():
                for _ in g:
                    pass
            fin()  # normalize the last attend
            # tail: q-half-1 pass B, head 3 rows only (head 2 already in yA)
            for _ in outproj_units(1, 1, rows=(D, P), mode="store"):
                pass
    _split_waits(nc)
    return nc


_PERM = None


def _perm64():
    global _PERM
    if _PERM is None:
        p = np.empty(D, dtype=np.int64)
        p[0::2] = np.arange(32)
        p[1::2] = np.arange(32) + 32
        _PERM = p
    return _PERM


def _rope_tables():
    perm = _perm64()
    inv_freq = 1.0 / (10000.0 ** (np.arange(0, D, 2, dtype=np.float32) / D))
    t = np.arange(T, dtype=np.float32)
    freqs = t[:, None] * inv_freq[None, :]  # [T, 32]
    emb = np.concatenate((freqs, freqs), axis=-1)  # [T, 64]
    cos = np.cos(emb).T.astype(np.float32)[perm]  # [64, T], permuted
    sin = np.sin(emb).T.astype(np.float32)[perm]
    sign = np.where(np.arange(D) % 2 == 0, -1.0, 1.0).astype(np.float32)
    sin_signed = sin * sign[:, None]
    # pre-pairswap: kernel computes rot = pairswap(q * sin_host), wants
    # rot[p] = q[p^1] * sin_signed[p], so sin_host[p] = sin_signed[p^1]
    sin_host = sin_signed[[p ^ 1 for p in range(D)]]
    cos2 = np.ascontiguousarray(np.concatenate([cos, cos], axis=0))  # [128,T]
    sin2 = np.ascontiguousarray(np.concatenate([sin_host, sin_host], 0))
    return cos2, sin2


def make_in_maps(x, wq, wk, wv, wo):
    import ml_dtypes

    bf = ml_dtypes.bfloat16
    perm = _perm64()
    cos2, sin2 = _rope_tables()

    def permute_heads(w):  # [256, 1024] -> row-permuted within each head
        return w.reshape(HPC, D, HIDDEN)[:, perm, :].reshape(HD, HIDDEN)

    in_maps = []
    for core in range(N_CORES):
        b, g = divmod(core, N_CORES // B)
        hs = slice(g * HD, (g + 1) * HD)
        in_maps.append(
            {
                "xT": np.ascontiguousarray(x[b].T).astype(bf),
                "wq_t": np.ascontiguousarray(permute_heads(wq[hs]).T).astype(bf),
                "wk_t": np.ascontiguousarray(permute_heads(wk[hs]).T).astype(bf),
                "wv_t": np.ascontiguousarray(wv[hs].T).astype(bf),
                "wo_t": np.ascontiguousarray(wo[:, hs].T).astype(bf),
                "cos2": cos2.astype(bf),
                "sin2": sin2.astype(bf),
            }
        )
    return in_maps


def gather_output(results):
    y = np.zeros((B, T, HIDDEN), dtype=np.float32)
    for core, res in enumerate(results):
        b = core // (N_CORES // B)
        y[b] += res["yT"].T.astype(np.float32)
    return y


_NC = None


def kernel(x, wq, wk, wv, wo):
    global _NC
    import time

    from concourse.bass_utils import run_bass_kernel_spmd

    if _NC is None:
        _NC = build_kernel()
    in_maps = make_in_maps(
        np.asarray(x), np.asarray(wq), np.asarray(wk), np.asarray(wv), np.asarray(wo)
    )
    try:
        res = run_bass_kernel_spmd(_NC, in_maps, core_ids=list(range(N_CORES)))
    except Exception:
        # transient device wedge (e.g. NRT_EXEC_UNIT_UNRECOVERABLE from a
        # prior run) -- retry once
        time.sleep(2.0)
        res = run_bass_kernel_spmd(_NC, in_maps, core_ids=list(range(N_CORES)))
    return gather_output(res.results)


# revision 48
# speedup vs baseline: 1.0454x; 1.0032x over previous
"""Self-contained Trainium2 Bass kernel for nn_Attention_41472204210330.

Multi-head attention (B=2, T=2048, HIDDEN=1024, 16 heads, head_dim=64, fp32)
with RoPE, sharded over 8 NeuronCores: data-parallel over the batch (2) x
tensor-parallel over heads (4 groups of 4 heads).  Each core computes its
batch's q/k/v projections for its 4 heads, RoPE, attention, and a partial
output projection (its heads' slice of wo); the host sums the 4 partials per
batch element.

Performance design (v2): the PE p-state/HAM clock only stays at full speed
(2.4 GHz) while the PE instruction stream is gap-free, so the whole kernel is
emitted as ONE dense PE schedule:
  - xT is DMA'd in 16 [128,1024] pieces in priority order so the first
    projection matmul issues ~1.6us in; everything else preloads behind it.
  - RoPE: the head-dim axis is interleaved host-side ([d0,d32,d1,d33,...]) so
    rotate-half becomes an intra-32 partition pair swap == one DVE
    stream_shuffle (no SBUF-SBUF DMA bounce).  cos/sin tables are bf16 with
    the sign pre-baked.
  - attends are ACT(exp)-bound by ~25%; a master filler sequence (v tiles,
    m=1 q/k projections, and the output projection split into per-head-pair
    accumulation passes A/B) drips PE work into every attend window.
  - softmax denominator via a ones-column in V; normalization is
    reciprocal([1,T] direct from PSUM) + gpsimd partition_broadcast (no DRAM
    bounce).
  - yT partials are stored in bf16 (host sums in fp32).
"""

import sys

if "/opt/trn_rl_repo" not in sys.path:
    sys.path.insert(0, "/opt/trn_rl_repo")

import numpy as np

import bass_rust
import concourse.bass as bass
import concourse.mybir as mybir
import concourse.tile as tile

HIDDEN = 1024
NUM_HEADS = 16
D = 64  # head dim
B = 2
T = 2048
N_CORES = 8
HPC = NUM_HEADS // (N_CORES // B)  # heads per core = 4
HD = HPC * D  # per-core head dims = 256
P = 128
F32 = mybir.dt.float32
BF16 = mybir.dt.bfloat16
NQ = 1024  # q-half width
IC_CH = HIDDEN // P  # 8 input-channel chunks
NKT = T // P  # 16 k tiles
VW = D + 1  # v columns per head incl. ones column


def _split_waits(nc):
    """The in-container walrus caps semaphore waits per instruction lower
    than bass_rust/Tile assume ("Too many sync wait commands").  Hoist all
    but one semaphore wait per instruction onto nop instructions inserted
    just before it in the same engine's program order (semantically
    identical: all waits still complete before the instruction runs)."""
    from concourse._compat import not_none

    def make_nop(engine, wait):
        nop = nc.engines[engine].nop(nofuse=True)
        nop.ins.sync_info = bass_rust.SyncInfo(on_wait=[wait], on_update=[])
        return nop.ins

    tail_bb = not_none(nc.cur_bb).bb
    plans = []
    for fn in nc.m.functions:
        for bb in fn.blocks:
            plan = {}
            for inst in bb.instructions:
                si = inst.sync_info
                waits = list(si.on_wait) if si and si.on_wait else []
                sem = [w for w in waits if w.sync_type == "semaphore"]
                if len(sem) > 1:
                    plan[inst.name] = sem[:-1]
            if plan:
                plans.append((bb, plan))
    created = {}
    n_tail_before = len(tail_bb.instructions)
    for bb, plan in plans:
        eng_of = {i.name: i.engine for i in bb.instructions}
        for iname, hoists in plan.items():
            created[iname] = [make_nop(eng_of[iname], w) for w in hoists]
    created_names = {n.name for nops in created.values() for n in nops}
    tail_insts = [i for i in tail_bb.instructions if i.name not in created_names]
    assert len(tail_insts) == n_tail_before
    tail_bb.instructions = tail_insts
    for bb, plan in plans:
        out = []
        for inst in bb.instructions:
            if inst.name in plan:
                hoisted = plan[inst.name]
                out.extend(created[inst.name])
                si = inst.sync_info
                si.on_wait = [w for w in si.on_wait if w not in hoisted]
            out.append(inst)
        bb.instructions = out


def build_kernel():
    nc = bass.Bass("TRN2", target_bir_lowering=False, debug=False)

    xT = nc.dram_tensor("xT", [HIDDEN, T], BF16, kind="ExternalInput")
    wq_t = nc.dram_tensor("wq_t", [HIDDEN, HD], BF16, kind="ExternalInput")
    wk_t = nc.dram_tensor("wk_t", [HIDDEN, HD], BF16, kind="ExternalInput")
    wv_t = nc.dram_tensor("wv_t", [HIDDEN, HD], BF16, kind="ExternalInput")
    wo_t = nc.dram_tensor("wo_t", [HD, HIDDEN], BF16, kind="ExternalInput")
    cos2 = nc.dram_tensor("cos2", [P, T], BF16, kind="ExternalInput")
    sin2 = nc.dram_tensor("sin2", [P, T], BF16, kind="ExternalInput")
    yT = nc.dram_tensor("yT", [HIDDEN, T], BF16, kind="ExternalOutput")

    mm = nc.tensor.matmul
    PAIRSWAP = [i ^ 1 for i in range(32)]

    with tile.TileContext(nc) as tc:
        with (
            nc.allow_low_precision(
                reason="bf16 matmul operands (fp32 PSUM accumulation); "
                "resid_var ~4e-5 end to end"
            ),
            tc.tile_pool(name="persist", bufs=1) as persist,
            tc.tile_pool(name="pt_pool", bufs=4) as pt_pool,
            tc.tile_pool(name="nrm_pool", bufs=2) as nrm_pool,
            tc.tile_pool(name="p1", bufs=3) as p1,
            tc.tile_pool(name="p3", bufs=4) as p3,
            # All PSUM comes from one shared pool (4 slots x 2 banks = all 8
            # banks) so phases pipeline without pool-boundary barriers.
            tc.tile_pool(name="psum", bufs=4, space="PSUM") as psum,
            tc.tile_pool(name="dram_pool", bufs=4, space="DRAM") as dram_pool,
        ):
            # ---- persistent SBUF tensors --------------------------------
            # All streaming tensors are split into [P, NQ] half-tiles:
            # dependency tracking is PER TILE, so whole-[P,T] tiles would
            # serialize the interleaved schedule with false WAR/RAW waits.
            qTr = [
                [
                    persist.tile([P, NQ], BF16, tag=f"qTr{m}_{n}", name=f"qTr{m}_{n}")
                    for n in range(2)
                ]
                for m in range(2)
            ]
            kTr = [
                [
                    persist.tile([P, NQ], BF16, tag=f"kTr{m}_{n}", name=f"kTr{m}_{n}")
                    for n in range(2)
                ]
                for m in range(2)
            ]
            v_sb = [
                persist.tile([P, HPC, VW], BF16, tag=f"v{kt}", name=f"v{kt}")
                for kt in range(NKT)
            ]
            otn = [
                [
                    persist.tile([P, NQ], BF16, tag=f"otn{m}_{n}", name=f"otn{m}_{n}")
                    for n in range(2)
                ]
                for m in range(2)
            ]
            # passA output-projection partials (bf16), one per q-half
            yA = [
                persist.tile([P, 8, NQ], BF16, tag=f"yA{n}", name=f"yA{n}")
                for n in range(2)
            ]
            wo_sb = persist.tile([P, 2, HIDDEN], BF16, tag="wo_sb", name="wo_sb")
            cos_sb = [
                persist.tile([P, NQ], BF16, tag=f"cos{n}", name=f"cos{n}")
                for n in range(2)
            ]
            sin_sb = [
                persist.tile([P, NQ], BF16, tag=f"sin{n}", name=f"sin{n}")
                for n in range(2)
            ]
            w_sbs = {}
            for name in ("q", "k", "v"):
                w_sbs[name] = persist.tile(
                    [P, IC_CH, HD], BF16, tag=f"w_{name}", name=f"w_{name}"
                )
            x_sb = [
                [
                    persist.tile([P, NQ], BF16, tag=f"x{c}_{t}", name=f"x{c}_{t}")
                    for t in range(2)
                ]
                for c in range(IC_CH)
            ]

            # ---- DMA preloads, priority order on the sync ring ----------
            nc.sync.dma_start(
                out=w_sbs["k"][:], in_=wk_t.rearrange("(c p) o -> p c o", p=P)
            )
            nc.sync.dma_start(
                out=w_sbs["q"][:], in_=wq_t.rearrange("(c p) o -> p c o", p=P)
            )
            nc.sync.dma_start(out=cos_sb[0][:], in_=cos2[:, 0:NQ])
            nc.sync.dma_start(out=sin_sb[0][:], in_=sin2[:, 0:NQ])
            for c in range(IC_CH):
                nc.sync.dma_start(
                    out=x_sb[c][0][:], in_=xT[c * P : (c + 1) * P, 0:NQ]
                )
            nc.sync.dma_start(
                out=w_sbs["v"][:], in_=wv_t.rearrange("(c p) o -> p c o", p=P)
            )
            for c in range(IC_CH):
                nc.sync.dma_start(
                    out=x_sb[c][1][:], in_=xT[c * P : (c + 1) * P, NQ:T]
                )
            nc.sync.dma_start(out=cos_sb[1][:], in_=cos2[:, NQ:T])
            nc.sync.dma_start(out=sin_sb[1][:], in_=sin2[:, NQ:T])
            nc.sync.dma_start(
                out=wo_sb[:], in_=wo_t.rearrange("(c p) o -> p c o", p=P)
            )
            for kt in range(NKT):
                nc.vector.memset(
                    v_sb[kt][:, :, D : D + 1].bitcast(mybir.dt.uint16),
                    0x3F80,  # bf16 bits of 1.0
                )
            # warm the ACT engine's Exp/Ln table during the DMA preload so
            # the first real exp doesn't pay the ACT_TABLE_LOAD
            warm_in = p1.tile([1, 64], F32, tag="warm_in", name="warm_in")
            warm_out = p1.tile([1, 64], BF16, tag="warm_out", name="warm_out")
            nc.vector.memset(warm_in[:], 0.0)
            nc.scalar.activation(
                out=warm_out[:],
                in_=warm_in[:],
                func=mybir.ActivationFunctionType.Exp,
            )

            # ---- q/k projection of one [128, NQ] tile + RoPE -------------
            def proj_qk_tile(name, m, n):
                """Generator: 8 c-chunk matmuls (yield per 2) + RoPE.

                RoPE runs per 512-column sub-tile so its DVE chain overlaps
                the second sub-tile's matmuls: dst = q*cos +
                pairswap(q*sin_host), sin_host sign-baked and pre-pairswapped
                on host so the shuffle runs SBUF->SBUF.
                """
                w_sb = w_sbs[name]
                dst = (qTr if name == "q" else kTr)[m][n]
                ps = psum.tile([P, NQ], F32, tag="main", name="ps")
                for sub in range(2):
                    ss = slice(sub * 512, (sub + 1) * 512)
                    for c in range(IC_CH):
                        mm(
                            ps[:, ss],
                            w_sb[:, c, m * P : (m + 1) * P],
                            x_sb[c][n][:, ss],
                            start=(c == 0),
                            stop=(c == IC_CH - 1),
                        )
                        if c % 4 == 3 and not (sub == 1 and c == IC_CH - 1):
                            yield
                    tmp = p1.tile([P, 512], BF16, tag="tmp", name="tmp")
                    nc.vector.tensor_mul(
                        out=tmp[:], in0=ps[:, ss], in1=sin_sb[n][:, ss]
                    )
                    rot = p1.tile([P, 512], BF16, tag="rot", name="rot")
                    nc.vector.stream_shuffle(
                        out=rot[:], in_=tmp[:], mask=PAIRSWAP
                    )
                    nc.vector.tensor_mul(
                        out=dst[:, ss], in0=ps[:, ss], in1=cos_sb[n][:, ss]
                    )
                    nc.gpsimd.tensor_add(
                        out=dst[:, ss], in0=dst[:, ss], in1=rot[:]
                    )
                yield

            # ---- v projection, one k-tile per unit -----------------------
            def proj_v_units():
                for kt in range(NKT):
                    t, tc_ = divmod(kt, 8)
                    psv = psum.tile([P, HPC, D], F32, tag="main", name="psv")
                    for c in range(IC_CH):
                        mm(
                            psv[:, :, :],
                            x_sb[c][t][:, tc_ * P : (tc_ + 1) * P],
                            w_sbs["v"][:, c, :],
                            start=(c == 0),
                            stop=(c == IC_CH - 1),
                        )
                    nc.vector.tensor_copy(
                        out=v_sb[kt][:, :, 0:D], in_=psv[:, :, :]
                    )
                    yield

            # ---- output projection, pass-split over head-pairs -----------
            def outproj_units(hf, cpass, rows=(0, P), mode=None):
                """One unit per 128-row output chunk mo: 2 sub-matmuls over
                head-pair `cpass` (contraction rows `rows`) + eviction.
                mode: 'toA' -> copy into yA; 'accA' -> add into yA in place;
                'store' -> add yA + store to yT."""
                if mode is None:
                    mode = "toA" if cpass == 0 else "store"
                for mo in range(HIDDEN // P):
                    ps = psum.tile([P, NQ], F32, tag="main", name="psy")
                    for sub in range(2):
                        ss = slice(sub * 512, (sub + 1) * 512)
                        mm(
                            ps[:, ss],
                            wo_sb[rows[0] : rows[1], cpass, mo * P : (mo + 1) * P],
                            otn[cpass][hf][rows[0] : rows[1], ss],
                            start=True,
                            stop=True,
                        )
                    if mode == "toA":
                        nc.vector.tensor_copy(out=yA[hf][:, mo, :], in_=ps[:])
                    elif mode == "accA":
                        nc.vector.tensor_add(
                            out=yA[hf][:, mo, :], in0=ps[:], in1=yA[hf][:, mo, :]
                        )
                    else:
                        ysb = p3.tile([P, NQ], BF16, tag="ysb", name="ysb")
                        nc.vector.tensor_add(
                            out=ysb[:], in0=ps[:], in1=yA[hf][:, mo, :]
                        )
                        nc.sync.dma_start(
                            out=yT[mo * P : (mo + 1) * P, hf * NQ : (hf + 1) * NQ],
                            in_=ysb[:],
                        )
                    yield

            # ---- attention for one (head, q-half) ------------------------
            LAG = 2  # attnv trails exp by 2 k-tiles so it never waits on ACT
            FIN_SLOT = 4  # kt at which the previous attend's norm finishes

            def attend(h, hf, pulls, master, finish_prev=None):
                m = h // 2
                r0 = (h % 2) * D
                # [P, NQ] = 2 PSUM banks: rows 0..63 = O accumulator, row 64
                # = exp-sum (ones column), rows 64..127 reused afterwards for
                # the broadcast reciprocal (no extra PSUM slot needed).
                ot = psum.tile([P, NQ], F32, tag="main", name="ot")
                pts = {}

                def attnv(j):
                    pt = pts.pop(j)
                    for sub in range(2):
                        ss = slice(sub * 512, (sub + 1) * 512)
                        mm(
                            ot[0:VW, ss],
                            v_sb[j][:, h, :],
                            pt[:, ss],
                            start=(j == 0),
                            stop=(j == NKT - 1),
                        )

                for kt in range(NKT):
                    kn, kc = divmod(kt, 8)
                    pt = pts[kt] = pt_pool.tile([P, NQ], BF16, tag="pt", name="pt")
                    st = psum.tile([P, NQ], F32, tag="main", name="st")
                    for sub in range(2):
                        q0 = sub * 512
                        mm(
                            st[:, sub * 512 : (sub + 1) * 512],
                            kTr[m][kn][r0 : r0 + D, kc * P : (kc + 1) * P],
                            qTr[m][hf][r0 : r0 + D, q0 : q0 + 512],
                            start=True,
                            stop=True,
                        )
                    if kt == FIN_SLOT and finish_prev is not None:
                        finish_prev()
                    for _ in range(pulls[kt] if kt < len(pulls) else 0):
                        next(master, None)
                    nc.scalar.activation(
                        out=pt[:],
                        in_=st[:],
                        func=mybir.ActivationFunctionType.Exp,
                        scale=float(1.0 / np.sqrt(D)),
                    )
                    if kt >= LAG:
                        attnv(kt - LAG)
                for j in range(NKT - LAG, NKT):
                    for _ in range(pulls[j + LAG] if j + LAG < len(pulls) else 0):
                        next(master, None)
                    attnv(j)
                # softmax denominator reciprocal without any table switch or
                # DRAM bounce: 1/den = exp(-ln(den)), both funcs live in the
                # same ACT table as the score exp.  The broadcast + multiply
                # are deferred into the next attend (finish()).
                ln_row = nrm_pool.tile([1, NQ], F32, tag="lnr", name="lnr")
                nc.scalar.activation(
                    out=ln_row[:],
                    in_=ot[D : D + 1, :],
                    func=mybir.ActivationFunctionType.Ln,
                )
                inv_bf = nrm_pool.tile([1, NQ], BF16, tag="invd", name="invd")
                nc.scalar.activation(
                    out=inv_bf[:],
                    in_=ln_row[:],
                    func=mybir.ActivationFunctionType.Exp,
                    scale=-1.0,
                )
                # broadcast 1/den across partitions via a DRAM bounce on the
                # sync ring (stride-0 partition reads need a DRAM source);
                # both DMAs run during the attend boundary, off every engine.
                dinv = dram_pool.tile([1, NQ], BF16, tag="dinv", name="dinv")
                nc.sync.dma_start(out=dinv[:], in_=inv_bf[:])
                rb = nrm_pool.tile([D, NQ], BF16, tag="rb", name="rb")
                src = dinv[0:1, :]
                nc.sync.dma_start(
                    out=rb[:],
                    in_=bass.AP(
                        tensor=src.tensor,
                        offset=src.offset,
                        ap=[[0, D]] + [list(a) for a in src.ap[1:]],
                    ),
                )

                def finish():
                    nc.vector.tensor_mul(
                        out=otn[m][hf][r0 : r0 + D, :],
                        in0=ot[0:D, :],
                        in1=rb[:],
                    )

                return finish

            # ---- emission schedule --------------------------------------
            # upfront: k/q projections (m=0, n=0) and v[0..2] (the PE is
            # otherwise idle while the RoPE chain drains on DVE/gpsimd).
            for _ in proj_qk_tile("k", 0, 0):
                pass
            for _ in proj_qk_tile("q", 0, 0):
                pass
            vgen = proj_v_units()
            for _ in range(3):
                next(vgen)

            gens = {
                "V": vgen,
                "K01": proj_qk_tile("k", 0, 1),
                "Q01": proj_qk_tile("q", 0, 1),
                "K10": proj_qk_tile("k", 1, 0),
                "Q10": proj_qk_tile("q", 1, 0),
                "K11": proj_qk_tile("k", 1, 1),
                "Q11": proj_qk_tile("q", 1, 1),
                "A0": outproj_units(0, 0),
                "A1": outproj_units(1, 0),
                "B0": outproj_units(0, 1),
                # q-half-1 pass B split by head so its first half fills the
                # last attend: head 2 accumulates into yA in place, head 3
                # (B2b below) is the only true tail.
                "B1a": outproj_units(1, 1, rows=(0, D), mode="accA"),
            }
            master_order = (
                ["V", "K01", "K01", "V"]                   # A1: 17
                + ["K01"] * 2 + ["V"] * 11
                + ["Q01"] * 4 + ["K10"] * 2                # A2: 6
                + ["K10"] * 2 + ["Q10"] * 4                # A3: 6
                + ["A0"] * 2 + ["K11"] * 4 + ["A0"] * 2    # A4: 8
                + ["A0"] * 4 + ["A1"] * 2                  # A5: 6
                + ["Q11"] * 4 + ["A1"] * 3                 # A6: 7
                + ["A1"] * 3 + ["B0"] * 3                  # A7: 6
                + ["B0"] * 1 + ["B1a"] * 8                 # A8: 9
            )
            # B0's remaining 4 units are deliberately NOT pulled: the safety
            # drain below emits them right after the last attend, keeping the
            # PE warm through the final normalization chain.

            def master():
                for sym in master_order:
                    yield next(gens[sym], None)

            ms = master()
            # filler pulls per kt slot (18 slots: 16 kts + 2 attnv-tail
            # slots).  After each attend boundary, pulls only start at kt>=3:
            # until the previous attend's deferred norm (FIN_SLOT) frees its
            # ot PSUM slot, all four psum slots are live.
            SP = {
                0: [2, 2, 2, 1, 1, 1, 1, 1, 1, 1, 1, 1, 1, 1, 0, 0, 0, 0],  # 17
                1: [0, 0, 0, 0, 0, 0, 1, 1, 0, 1, 0, 1, 0, 1, 0, 0, 1, 0],  # 6
                2: [0, 0, 0, 0, 0, 0, 1, 1, 0, 1, 0, 1, 0, 1, 0, 0, 1, 0],  # 6
                3: [0, 0, 0, 0, 0, 0, 1, 1, 1, 1, 1, 1, 0, 1, 0, 0, 1, 0],  # 8
                4: [0, 0, 0, 0, 0, 0, 1, 1, 0, 1, 0, 1, 0, 1, 0, 0, 1, 0],  # 6
                5: [0, 0, 0, 0, 0, 0, 1, 1, 1, 1, 0, 1, 0, 1, 0, 0, 1, 0],  # 7
                6: [0, 0, 0, 0, 0, 0, 1, 1, 0, 1, 0, 1, 0, 1, 0, 0, 1, 0],  # 6
                7: [0, 0, 0, 0, 0, 0, 1, 1, 1, 1, 1, 1, 1, 1, 0, 0, 1, 0],  # 9
            }
            order = [(0, 0), (1, 0), (0, 1), (1, 1), (2, 0), (3, 0), (2, 1), (3, 1)]
            fin = None
            for i, (h, hf) in enumerate(order):
                fin = attend(h, hf, SP[i], ms, finish_prev=fin)
            for _ in ms:  # drain any master remainder
                pass
            # B0's reserved store units keep the PE warm while the last
            # attend's norm completes on the ACT engine.
            for g in gens.values():
                for _ in g:
                    pass
            fin()  # normalize the last attend
            # tail: q-half-1 pass B, head 3 rows only (head 2 already in yA)
            for _ in outproj_units(1, 1, rows=(D, P), mode="store"):
                pass
    _split_waits(nc)
    return nc


_PERM = None


def _perm64():
    global _PERM
    if _PERM is None:
        p = np.empty(D, dtype=np.int64)
        p[0::2] = np.arange(32)
        p[1::2] = np.arange(32) + 32
        _PERM = p
    return _PERM


def _rope_tables():
    perm = _perm64()
    inv_freq = 1.0 / (10000.0 ** (np.arange(0, D, 2, dtype=np.float32) / D))
    t = np.arange(T, dtype=np.float32)
    freqs = t[:, None] * inv_freq[None, :]  # [T, 32]
    emb = np.concatenate((freqs, freqs), axis=-1)  # [T, 64]
    cos = np.cos(emb).T.astype(np.float32)[perm]  # [64, T], permuted
    sin = np.sin(emb).T.astype(np.float32)[perm]
    sign = np.where(np.arange(D) % 2 == 0, -1.0, 1.0).astype(np.float32)
    sin_signed = sin * sign[:, None]
    # pre-pairswap: kernel computes rot = pairswap(q * sin_host), wants
    # rot[p] = q[p^1] * sin_signed[p], so sin_host[p] = sin_signed[p^1]
    sin_host = sin_signed[[p ^ 1 for p in range(D)]]
    cos2 = np.ascontiguousarray(np.concatenate([cos, cos], axis=0))  # [128,T]
    sin2 = np.ascontiguousarray(np.concatenate([sin_host, sin_host], 0))
    return cos2, sin2


def make_in_maps(x, wq, wk, wv, wo):
    import ml_dtypes

    bf = ml_dtypes.bfloat16
    perm = _perm64()
    cos2, sin2 = _rope_tables()

    def permute_heads(w):  # [256, 1024] -> row-permuted within each head
        return w.reshape(HPC, D, HIDDEN)[:, perm, :].reshape(HD, HIDDEN)

    in_maps = []
    for core in range(N_CORES):
        b, g = divmod(core, N_CORES // B)
        hs = slice(g * HD, (g + 1) * HD)
        in_maps.append(
            {
                "xT": np.ascontiguousarray(x[b].T).astype(bf),
                "wq_t": np.ascontiguousarray(permute_heads(wq[hs]).T).astype(bf),
                "wk_t": np.ascontiguousarray(permute_heads(wk[hs]).T).astype(bf),
                "wv_t": np.ascontiguousarray(wv[hs].T).astype(bf),
                "wo_t": np.ascontiguousarray(wo[:, hs].T).astype(bf),
                "cos2": cos2.astype(bf),
                "sin2": sin2.astype(bf),
            }
        )
    return in_maps


def gather_output(results):
    y = np.zeros((B, T, HIDDEN), dtype=np.float32)
    for core, res in enumerate(results):
        b = core // (N_CORES // B)
        y[b] += res["yT"].T.astype(np.float32)
    return y


_NC = None


def kernel(x, wq, wk, wv, wo):
    global _NC
    import time

    from concourse.bass_utils import run_bass_kernel_spmd

    if _NC is None:
        _NC = build_kernel()
    in_maps = make_in_maps(
        np.asarray(x), np.asarray(wq), np.asarray(wk), np.asarray(wv), np.asarray(wo)
    )
    try:
        res = run_bass_kernel_spmd(_NC, in_maps, core_ids=list(range(N_CORES)))
    except Exception:
        # transient device wedge (e.g. NRT_EXEC_UNIT_UNRECOVERABLE from a
        # prior run) -- retry once
        time.sleep(2.0)
        res = run_bass_kernel_spmd(_NC, in_maps, core_ids=list(range(N_CORES)))
    return gather_output(res.results)
